# revision 12
# baseline (speedup 1.0000x reference)
# Trainium2 Bass/Tile kernel for a single-layer transformer encoder block:
#   q,k,v = x@Wq, x@Wk, x@Wv  (16-head attention, no mask, no out-proj)
#   x1  = LN(x + attn);  out = LN(x1 + relu(x1@W1+b1)@W2+b2)
#
# Sharding: 8 cores = (batch b in 0..3) x (sequence half in 0..1).
# Each core computes full-sequence K/V for its batch item (needed by both
# halves) and everything else for its 512-token query half. No collectives.
#
# The host passes xT = x[b].T with the core's query-half columns FIRST
# (key order is softmax-invariant), so one program serves all 8 cores and
# no fp32 transpose of x is ever needed on-chip. Host also pre-casts
# weights and xT to bf16 (halves weight DMA; matmuls run at 1 cyc/row).
#
# Layouts (hardcoded for B=4, T=1024, D=1024, H=16, DK=64, DF=4096):
#   - qT,kT = W.T @ xT (feature-major, bf16);  v = xT.T @ Wv (token-major)
#   - qT is stored zero-PADDED per head [128, H, TQ]: head h occupies
#     partitions (h%2)*64..+64, the other 64 partitions are zero. Score
#     matmuls then contract over the full K=128 (identical cycle count,
#     but keeps the PE HAM activity monitor warm -> 2.4 GHz, not 1.2).
#   - scores^T per head: S^T[tk,tq] = kT(pair).T @ qT_pad_h
#   - P^T = exp(S^T/8) via ACT psum->sbuf (bf16)
#   - att^T[65,tq] = [v_h | 1].T @ P^T  (row 64 = softmax denominator)
#   - PE-transpose att^T back to token-major, normalize via DVE reciprocal
#   - FFN: hT[df,tq] = relu(W1.T @ x1T + b1) (ACT relu psum->bf16),
#     ff[tq,do] = hT.T @ W2 accumulated over all 32 df-chunks.
#   - W1/W2 stream pools are allocated over the released QKV-phase region
#     so their prefetch DMAs run during attention, not after LN1.
import ml_dtypes
import numpy as np

import concourse.bass as bass
import concourse.tile as tile
from concourse import bacc, mybir
from concourse.bass import ts
from concourse.bass_utils import run_bass_kernel_spmd
from concourse.masks import make_identity

F32 = mybir.dt.float32
BF16 = mybir.dt.bfloat16
AF = mybir.ActivationFunctionType
ALU = mybir.AluOpType

B, T, D, H, DK, DF = 4, 1024, 1024, 16, 64, 4096
EPS = 1e-5
P = 128
TQ = 512          # query tokens per core
NTQ = TQ // P     # 4 query tiles
NTK = T // P      # 8 key tiles
NDC = D // P      # 8 contraction chunks of D
NDF = DF // P     # 32 df tiles
N_CORES = 8


def _layer_norm_inplace(nc, stats_pool, small_pool, x_io, g_bc, b_bc, eps_sb):
    """In-place LN over the last (free) dim of x_io [128, NTQ, D], with
    per-feature gain/bias tiles g_bc/b_bc [128, D] (partition-broadcast)."""
    for q in range(NTQ):
        xi = x_io[:, q, :]
        stats = stats_pool.tile([P, 2, 6], F32, tag="ln_stats")
        for s in range(2):
            nc.vector.bn_stats(out=stats[:, s, :], in_=xi[:, ts(s, 512)])
        mv = stats_pool.tile([P, 2], F32, tag="ln_mv")
        nc.vector.bn_aggr(out=mv, in_=stats)
        std = small_pool.tile([P, 1], F32, tag="ln_std")
        nc.scalar.activation(out=std, in_=mv[:, 1:2], func=AF.Sqrt, bias=eps_sb)
        rstd = small_pool.tile([P, 1], F32, tag="ln_rstd")
        nc.vector.reciprocal(out=rstd, in_=std)
        nc.vector.tensor_scalar(
            out=xi, in0=xi, scalar1=mv[:, 0:1], scalar2=rstd,
            op0=ALU.subtract, op1=ALU.mult,
        )
        nc.vector.tensor_mul(out=xi, in0=xi, in1=g_bc)
        nc.vector.tensor_add(out=xi, in0=xi, in1=b_bc)


def build_nc():
    nc = bacc.Bacc("TRN2", target_bir_lowering=False, debug=False)

    xT = nc.dram_tensor("xT", [D, T], BF16, kind="ExternalInput").ap()
    xh = nc.dram_tensor("xh", [TQ, D], F32, kind="ExternalInput").ap()
    Wq = nc.dram_tensor("Wq", [D, D], BF16, kind="ExternalInput").ap()
    Wk = nc.dram_tensor("Wk", [D, D], BF16, kind="ExternalInput").ap()
    Wv = nc.dram_tensor("Wv", [D, D], BF16, kind="ExternalInput").ap()
    W1 = nc.dram_tensor("W1", [D, DF], BF16, kind="ExternalInput").ap()
    b1 = nc.dram_tensor("b1", [DF], F32, kind="ExternalInput").ap()
    W2 = nc.dram_tensor("W2", [DF, D], BF16, kind="ExternalInput").ap()
    b2 = nc.dram_tensor("b2", [D], F32, kind="ExternalInput").ap()
    g1 = nc.dram_tensor("ln1_g", [D], F32, kind="ExternalInput").ap()
    be1 = nc.dram_tensor("ln1_b", [D], F32, kind="ExternalInput").ap()
    g2 = nc.dram_tensor("ln2_g", [D], F32, kind="ExternalInput").ap()
    be2 = nc.dram_tensor("ln2_b", [D], F32, kind="ExternalInput").ap()
    out = nc.dram_tensor("out", [TQ, D], F32, kind="ExternalOutput").ap()

    def bcast(ap_1d, n):
        return bass.AP(tensor=ap_1d.tensor, offset=ap_1d.offset, ap=[[0, P], [1, n]])

    with tile.TileContext(nc) as tc:
        with (
            tc.tile_pool(name="consts", bufs=1) as consts,
            tc.tile_pool(name="cpersist", bufs=1) as cpersist,
            tc.tile_pool(name="small", bufs=4) as small,
            tc.tile_pool(name="stats", bufs=4) as stats,
        ):
            ident = consts.tile([P, P], F32)
            make_identity(nc, ident)
            eps_sb = consts.tile([P, 1], F32)
            nc.vector.memset(eps_sb, EPS)

            # survive across phases (stack-allocated below everything else)
            att_sb = cpersist.tile([P, NTQ, H, DK], F32)     # 16 KB/part
            x1b_sb = cpersist.tile([P, NTQ, D], F32)         # x1 + b2, 16
            x1T_sb = cpersist.tile([P, NDC, TQ], BF16)       # 8

            # ================= Phase A+B: QKV + attention =================
            with tc.tile_pool(name="qkv_sb", bufs=1) as qkv_sb:
                # qT zero-padded per head: head h on partitions (h%2)*64..+64
                qT_pad = qkv_sb.tile([P, H, TQ], BF16)         # 16 KB/part
                nc.vector.memset(qT_pad, 0.0)
                kT_sb = qkv_sb.tile([P, NDC, T], BF16)         # 16
                v_sb = qkv_sb.tile([P, NTK, H, DK + 1], BF16)  # 16.25
                nc.vector.memset(v_sb[:, :, :, DK : DK + 1], 1.0)

                with (
                    tc.tile_pool(name="xT_pool", bufs=1) as xT_pool,
                    tc.tile_pool(name="wload", bufs=3) as wload,
                    tc.tile_pool(name="qkv_ps", bufs=4, space="PSUM") as qkv_ps,
                ):
                    xT_sb = xT_pool.tile([P, NDC, T], BF16)    # 16 KB/part
                    xTre = xT.rearrange("(c p) t -> p c t", p=P)
                    for dc in range(NDC):  # chunked so compute starts early
                        nc.sync.dma_start(out=xT_sb[:, dc, :], in_=xTre[:, dc, :])

                    for name, W in (("q", Wq), ("k", Wk), ("v", Wv)):
                        Wr = W.rearrange("(c p) o -> p c o", p=P)
                        for oc in range(4):  # 256-wide output chunks
                            w_t = wload.tile([P, NDC, 256], BF16, tag="wchunk")
                            nc.sync.dma_start(out=w_t, in_=Wr[:, :, ts(oc, 256)])
                            if name == "v":
                                # token-major v: lhsT = xT tile, rhs = Wv chunk
                                for tk in range(NTK):
                                    ps = qkv_ps.tile([P, 256], F32, tag="v")
                                    for dc in range(NDC):
                                        nc.tensor.matmul(
                                            ps,
                                            lhsT=xT_sb[:, dc, ts(tk, P)],
                                            rhs=w_t[:, dc, :],
                                            start=(dc == 0),
                                            stop=(dc == NDC - 1),
                                        )
                                    # 4 heads' [128, 64] slices (stride 65)
                                    nc.vector.tensor_copy(
                                        out=v_sb[:, tk, oc * 4 : oc * 4 + 4, 0:DK],
                                        in_=ps.rearrange("p (h d) -> p h d", d=DK),
                                    )
                            elif name == "q":
                                # feature-major q, zero-padded per head
                                for ot in range(2):  # 128-wide tiles in chunk
                                    o_tile = oc * 2 + ot
                                    ps = qkv_ps.tile([P, TQ], F32, tag="qk")
                                    for dc in range(NDC):
                                        nc.tensor.matmul(
                                            ps,
                                            lhsT=w_t[:, dc, ts(ot, P)],
                                            rhs=xT_sb[:, dc, 0:TQ],
                                            start=(dc == 0),
                                            stop=(dc == NDC - 1),
                                        )
                                    for i in range(2):  # head halves
                                        h = 2 * o_tile + i
                                        hp = i * DK
                                        nc.vector.tensor_copy(
                                            out=qT_pad[hp : hp + DK, h, :],
                                            in_=ps[hp : hp + DK, :],
                                        )
                            else:
                                # feature-major k
                                for ot in range(2):
                                    o_tile = oc * 2 + ot
                                    for tci in range(2):
                                        ps = qkv_ps.tile([P, TQ], F32, tag="qk")
                                        for dc in range(NDC):
                                            nc.tensor.matmul(
                                                ps,
                                                lhsT=w_t[:, dc, ts(ot, P)],
                                                rhs=xT_sb[:, dc, ts(tci, 512)],
                                                start=(dc == 0),
                                                stop=(dc == NDC - 1),
                                            )
                                        nc.vector.tensor_copy(
                                            out=kT_sb[:, o_tile, ts(tci, 512)],
                                            in_=ps,
                                        )

                # FFN stream pools sit on the region released by
                # xT_pool/wload above, so W1/W2 prefetch DMAs only wait for
                # the end of QKV and run during attention.
                with (
                    tc.tile_pool(name="c2_sb", bufs=1) as c2_sb,
                    tc.tile_pool(name="w1load", bufs=2) as w1load,
                    tc.tile_pool(name="w2load", bufs=2) as w2load,
                ):
                    b1_sb = c2_sb.tile([P, NDF], F32)
                    nc.sync.dma_start(
                        out=b1_sb, in_=b1.rearrange("(c p) -> p c", p=P)
                    )
                    g2_bc = c2_sb.tile([P, D], F32)
                    nc.sync.dma_start(out=g2_bc, in_=bcast(g2, D))
                    be2_bc = c2_sb.tile([P, D], F32)
                    nc.sync.dma_start(out=be2_bc, in_=bcast(be2, D))
                    hT_sb = c2_sb.tile([P, NDF, TQ], BF16)   # 32 KB/part

                    W1r = W1.rearrange("(c p) f -> p c f", p=P)
                    w1_tiles = []
                    for fc in range(8):  # 512-wide df chunks
                        w_t = w1load.tile(
                            [P, NDC, 512], BF16, tag="w1chunk", name=f"w1_{fc}"
                        )
                        nc.sync.dma_start(out=w_t, in_=W1r[:, :, ts(fc, 512)])
                        w1_tiles.append(w_t)
                    W2r = W2.rearrange("(c p) o -> p c o", p=P)
                    w2_tiles = []
                    for blk in range(8):  # 4 df-tiles per W2 block
                        w_t = w2load.tile(
                            [P, 4, D], BF16, tag="w2chunk", name=f"w2_{blk}"
                        )
                        nc.sync.dma_start(
                            out=w_t, in_=W2r[:, blk * 4 : blk * 4 + 4, :]
                        )
                        w2_tiles.append(w_t)

                    # ---- attention ----
                    with (
                        tc.tile_pool(name="pt_pool", bufs=2) as pt_pool,
                        tc.tile_pool(name="attT_pool", bufs=2) as attT_pool,
                        tc.tile_pool(name="st_ps", bufs=4, space="PSUM") as st_ps,
                        tc.tile_pool(name="att_ps", bufs=2, space="PSUM") as att_ps,
                        tc.tile_pool(name="tr_ps", bufs=2, space="PSUM") as tr_ps,
                    ):
                        for h in range(H):
                            hc = h // 2  # feature chunk of kT
                            pt = pt_pool.tile([P, NTK, TQ], BF16, tag="pt")
                            for tk in range(NTK):
                                st = st_ps.tile([P, TQ], F32, tag="st")
                                # kT holds the head pair; qT_pad is zero on
                                # the other head's partitions -> K=128
                                nc.tensor.matmul(
                                    st,
                                    lhsT=kT_sb[:, hc, ts(tk, P)],
                                    rhs=qT_pad[:, h, :],
                                    start=True,
                                    stop=True,
                                )
                                nc.scalar.activation(
                                    out=pt[:, tk, :], in_=st,
                                    func=AF.Exp, scale=0.125,
                                )
                            aps = att_ps.tile([DK + 1, TQ], F32, tag="attT")
                            for tk in range(NTK):
                                nc.tensor.matmul(
                                    aps,
                                    lhsT=v_sb[:, tk, h, :],
                                    rhs=pt[:, tk, :],
                                    start=(tk == 0),
                                    stop=(tk == NTK - 1),
                                )
                            attT = attT_pool.tile(
                                [DK + 1, TQ], F32, tag="attT_sb"
                            )
                            nc.vector.tensor_copy(out=attT, in_=aps)
                            for q in range(NTQ):
                                trp = tr_ps.tile([P, DK + 1], F32, tag="tr")
                                nc.tensor.transpose(
                                    trp,
                                    attT[:, ts(q, P)],
                                    ident[: DK + 1, : DK + 1],
                                )
                                rec = small.tile([P, 1], F32, tag="rec")
                                nc.vector.reciprocal(
                                    out=rec, in_=trp[:, DK : DK + 1]
                                )
                                nc.vector.tensor_scalar_mul(
                                    out=att_sb[:, q, h, :],
                                    in0=trp[:, 0:DK],
                                    scalar1=rec,
                                )

                    # ============ Phase C1: residual + LN1 + x1T ============
                    with (
                        tc.tile_pool(name="c1_sb", bufs=1) as c1_sb,
                        tc.tile_pool(name="c1_ps", bufs=2, space="PSUM") as c1_ps,
                    ):
                        g1_bc = c1_sb.tile([P, D], F32)
                        nc.sync.dma_start(out=g1_bc, in_=bcast(g1, D))
                        be1_bc = c1_sb.tile([P, D], F32)
                        nc.sync.dma_start(out=be1_bc, in_=bcast(be1, D))
                        b2_bc = c1_sb.tile([P, D], F32)
                        nc.sync.dma_start(out=b2_bc, in_=bcast(b2, D))

                        x1_sb = c1_sb.tile([P, NTQ, D], F32)
                        nc.sync.dma_start(
                            out=x1_sb, in_=xh.rearrange("(q p) d -> p q d", p=P)
                        )
                        nc.vector.tensor_add(
                            out=x1_sb,
                            in0=x1_sb,
                            in1=att_sb.rearrange("p q h d -> p q (h d)"),
                        )
                        _layer_norm_inplace(
                            nc, stats, small, x1_sb, g1_bc, be1_bc, eps_sb
                        )
                        # x1 + b2 (residual plus ff-output bias, used post-FFN)
                        nc.vector.tensor_add(
                            out=x1b_sb,
                            in0=x1_sb,
                            in1=b2_bc[:, None, :].to_broadcast((P, NTQ, D)),
                        )
                        for q in range(NTQ):
                            for dc in range(NDC):
                                trp = c1_ps.tile([P, P], F32, tag="x1t")
                                nc.tensor.transpose(
                                    trp, x1_sb[:, q, ts(dc, P)], ident
                                )
                                nc.vector.tensor_copy(
                                    out=x1T_sb[:, dc, ts(q, P)], in_=trp
                                )

                    # ================= Phase C2: FFN =================
                    with tc.tile_pool(name="h_ps", bufs=4, space="PSUM") as h_ps:
                        for fc in range(8):
                            w_t = w1_tiles[fc]
                            for ft in range(4):
                                df = fc * 4 + ft
                                ps = h_ps.tile([P, TQ], F32, tag="h")
                                for dc in range(NDC):
                                    nc.tensor.matmul(
                                        ps,
                                        lhsT=w_t[:, dc, ts(ft, P)],
                                        rhs=x1T_sb[:, dc, :],
                                        start=(dc == 0),
                                        stop=(dc == NDC - 1),
                                    )
                                nc.scalar.activation(
                                    out=hT_sb[:, df, :],
                                    in_=ps,
                                    func=AF.Relu,
                                    bias=b1_sb[:, df : df + 1],
                                )

                    with tc.tile_pool(name="ff_ps", bufs=1, space="PSUM") as ff_ps:
                        ff_tiles = [
                            [
                                ff_ps.tile(
                                    [P, 512], F32,
                                    tag=f"ff_{q}_{oc}", name=f"ff_{q}_{oc}",
                                )
                                for oc in range(2)
                            ]
                            for q in range(NTQ)
                        ]
                        for blk in range(8):
                            w_t = w2_tiles[blk]
                            for fr in range(4):
                                df = blk * 4 + fr
                                for q in range(NTQ):
                                    for oc in range(2):
                                        nc.tensor.matmul(
                                            ff_tiles[q][oc],
                                            lhsT=hT_sb[:, df, ts(q, P)],
                                            rhs=w_t[:, fr, ts(oc, 512)],
                                            start=(df == 0),
                                            stop=(df == NDF - 1),
                                        )
                        # ffa = ff + (x1 + b2), in place into x1b
                        for q in range(NTQ):
                            for oc in range(2):
                                nc.vector.tensor_add(
                                    out=x1b_sb[:, q, ts(oc, 512)],
                                    in0=ff_tiles[q][oc],
                                    in1=x1b_sb[:, q, ts(oc, 512)],
                                )

                    _layer_norm_inplace(
                        nc, stats, small, x1b_sb, g2_bc, be2_bc, eps_sb
                    )
            nc.sync.dma_start(
                out=out.rearrange("(q p) d -> p q d", p=P), in_=x1b_sb
            )

    nc.compile()
    return nc


_NC_CACHE = None


def _get_nc():
    global _NC_CACHE
    if _NC_CACHE is None:
        _NC_CACHE = build_nc()
    return _NC_CACHE


def _core_in_map(x, shared, c):
    b, half = divmod(c, 2)
    xb = x[b]
    sl = slice(half * TQ, (half + 1) * TQ)
    other = slice((1 - half) * TQ, (2 - half) * TQ)
    # query-half columns first; key order is softmax-invariant
    xT_perm = np.concatenate([xb[sl].T, xb[other].T], axis=1)
    return dict(
        shared,
        xT=np.ascontiguousarray(xT_perm.astype(ml_dtypes.bfloat16)),
        xh=np.ascontiguousarray(xb[sl]),
    )


def kernel(x, Wq, Wk, Wv, W1, b1, W2, b2, ln1_g, ln1_b, ln2_g, ln2_b):
    x = np.ascontiguousarray(np.asarray(x, dtype=np.float32))
    shared = {
        k: np.ascontiguousarray(np.asarray(v, np.float32))
        for k, v in dict(
            b1=b1, b2=b2,
            ln1_g=ln1_g, ln1_b=ln1_b, ln2_g=ln2_g, ln2_b=ln2_b,
        ).items()
    }
    shared.update(
        {
            k: np.ascontiguousarray(np.asarray(v).astype(ml_dtypes.bfloat16))
            for k, v in dict(Wq=Wq, Wk=Wk, Wv=Wv, W1=W1, W2=W2).items()
        }
    )
    in_maps = [_core_in_map(x, shared, c) for c in range(N_CORES)]
    nc = _get_nc()
    res = run_bass_kernel_spmd(nc, in_maps, core_ids=list(range(N_CORES)))
    out = np.empty((B, T, D), np.float32)
    for c in range(N_CORES):
        b, half = divmod(c, 2)
        out[b, half * TQ : (half + 1) * TQ] = res.results[c]["out"]
    return out


# revision 13
# speedup vs baseline: 1.0276x; 1.0276x over previous
# Trainium2 Bass/Tile kernel for a single-layer transformer encoder block:
#   q,k,v = x@Wq, x@Wk, x@Wv  (16-head attention, no mask, no out-proj)
#   x1  = LN(x + attn);  out = LN(x1 + relu(x1@W1+b1)@W2+b2)
#
# Sharding: 8 cores = (batch b in 0..3) x (sequence half in 0..1).
# Each core computes full-sequence K/V for its batch item (needed by both
# halves) and everything else for its 512-token query half. No collectives.
#
# Host-side preprocessing (pure input transforms):
#   - xT = x[b].T with the core's query-half columns FIRST (key order is
#     softmax-invariant) -> one program serves all 8 cores, no fp32
#     transpose of x on-chip.
#   - weights and xT pre-cast to bf16 (halves weight DMA; 1 cyc/row PE).
#   - LN1's gain/bias folded into the FFN: W1g = diag(ln1_g) @ W1 (bf16),
#     b1p = b1 + ln1_b @ W1, beb = ln1_b + b2. On-chip LN1 then only
#     normalizes (xan = (xa-mu)*rstd); the residual term x1+b2 =
#     xan*g1 + beb is computed off the critical path during the FFN.
#
# On-chip layouts (hardcoded for B=4, T=1024, D=1024, H=16, DK=64, DF=4096):
#   - qT,kT = W.T @ xT (feature-major, bf16); v = xT.T @ Wv (token-major)
#   - qT stored zero-PADDED per head [128, H, TQ] so score matmuls
#     contract over K=128 (same cycles, keeps the PE HAM monitor warm).
#   - P^T = exp(S^T/8) via ACT psum->sbuf (bf16)
#   - att^T[65,tq] = [v_h | 1].T @ P^T  (row 64 = softmax denominator);
#     PE-transpose back to token-major, normalize via DVE reciprocal.
#   - FFN: hT[df,tq] = relu(W1g.T @ xanT + b1p) (ACT relu psum->bf16);
#     ff[tq,do] = hT.T @ W2 in two half-passes; pass B is q-tile-major so
#     residual+LN2+output-DMA pipeline per tile while the PE finishes.
#   - queue-mode pool allocator + per-engine DMA queues so W1/W2 streams
#     prefetch during attention / FFN-h instead of serializing.
import ml_dtypes
import numpy as np

import concourse.bass as bass
import concourse.tile as tile
from concourse import bacc, mybir
from concourse.bass import ts
from concourse.bass_utils import run_bass_kernel_spmd
from concourse.masks import make_identity

F32 = mybir.dt.float32
BF16 = mybir.dt.bfloat16
AF = mybir.ActivationFunctionType
ALU = mybir.AluOpType

B, T, D, H, DK, DF = 4, 1024, 1024, 16, 64, 4096
EPS = 1e-5
P = 128
TQ = 512          # query tokens per core
NTQ = TQ // P     # 4 query tiles
NTK = T // P      # 8 key tiles
NDC = D // P      # 8 contraction chunks of D
NDF = DF // P     # 32 df tiles
N_CORES = 8


def _ln_normalize_tile(nc, stats_pool, small_pool, xi, eps_sb):
    """In-place (x - mean) * rsqrt(var + eps) over the free dim of
    xi [128, D]. Returns nothing; caller applies gain/bias if needed."""
    stats = stats_pool.tile([P, 2, 6], F32, tag="ln_stats")
    for s in range(2):
        nc.vector.bn_stats(out=stats[:, s, :], in_=xi[:, ts(s, 512)])
    mv = stats_pool.tile([P, 2], F32, tag="ln_mv")
    nc.vector.bn_aggr(out=mv, in_=stats)
    std = small_pool.tile([P, 1], F32, tag="ln_std")
    nc.scalar.activation(out=std, in_=mv[:, 1:2], func=AF.Sqrt, bias=eps_sb)
    rstd = small_pool.tile([P, 1], F32, tag="ln_rstd")
    nc.vector.reciprocal(out=rstd, in_=std)
    nc.vector.tensor_scalar(
        out=xi, in0=xi, scalar1=mv[:, 0:1], scalar2=rstd,
        op0=ALU.subtract, op1=ALU.mult,
    )


def build_nc():
    nc = bacc.Bacc("TRN2", target_bir_lowering=False, debug=False)

    xT = nc.dram_tensor("xT", [D, T], BF16, kind="ExternalInput").ap()
    xh = nc.dram_tensor("xh", [TQ, D], F32, kind="ExternalInput").ap()
    Wq = nc.dram_tensor("Wq", [D, D], BF16, kind="ExternalInput").ap()
    Wk = nc.dram_tensor("Wk", [D, D], BF16, kind="ExternalInput").ap()
    Wv = nc.dram_tensor("Wv", [D, D], BF16, kind="ExternalInput").ap()
    W1 = nc.dram_tensor("W1", [D, DF], BF16, kind="ExternalInput").ap()
    b1p = nc.dram_tensor("b1p", [DF], F32, kind="ExternalInput").ap()
    W2 = nc.dram_tensor("W2", [DF, D], BF16, kind="ExternalInput").ap()
    g1 = nc.dram_tensor("ln1_g", [D], F32, kind="ExternalInput").ap()
    beb = nc.dram_tensor("beb", [D], F32, kind="ExternalInput").ap()
    g2 = nc.dram_tensor("ln2_g", [D], F32, kind="ExternalInput").ap()
    be2 = nc.dram_tensor("ln2_b", [D], F32, kind="ExternalInput").ap()
    out = nc.dram_tensor("out", [TQ, D], F32, kind="ExternalOutput").ap()

    def bcast(ap_1d, n):
        return bass.AP(tensor=ap_1d.tensor, offset=ap_1d.offset, ap=[[0, P], [1, n]])

    with tile.TileContext(nc, pool_alloc_mode="queue") as tc:
        with (
            tc.tile_pool(name="consts", bufs=1) as consts,
            tc.tile_pool(name="cpersist", bufs=1) as cpersist,
            tc.tile_pool(name="small", bufs=4) as small,
            tc.tile_pool(name="stats", bufs=4) as stats,
        ):
            ident = consts.tile([P, P], F32)
            make_identity(nc, ident)
            eps_sb = consts.tile([P, 1], F32)
            nc.vector.memset(eps_sb, EPS)

            att_sb = cpersist.tile([P, NTQ, H, DK], F32)     # 16 KB/part
            x1b_sb = cpersist.tile([P, NTQ, D], F32)         # x1 + b2, 16
            x1T_sb = cpersist.tile([P, NDC, TQ], BF16)       # 8

            # ================= Phase A: QKV projections =================
            with tc.tile_pool(name="qkv_sb", bufs=1) as qkv_sb:
                # qT zero-padded per head: head h on partitions (h%2)*64..+64
                qT_pad = qkv_sb.tile([P, H, TQ], BF16)         # 16 KB/part
                nc.vector.memset(qT_pad, 0.0)
                kT_sb = qkv_sb.tile([P, NDC, T], BF16)         # 16
                v_sb = qkv_sb.tile([P, NTK, H, DK + 1], BF16)  # 16.25
                nc.vector.memset(v_sb[:, :, :, DK : DK + 1], 1.0)

                with (
                    tc.tile_pool(name="xT_pool", bufs=1) as xT_pool,
                    tc.tile_pool(name="wload", bufs=3) as wload,
                    tc.tile_pool(name="qkv_ps", bufs=4, space="PSUM") as qkv_ps,
                ):
                    # first Wq chunk before the bulk xT load so the PE can
                    # start as soon as the first xT chunks land
                    Wqr = Wq.rearrange("(c p) o -> p c o", p=P)
                    wq0 = wload.tile(
                        [P, NDC, 256], BF16, tag="wchunk", name="wq0"
                    )
                    nc.sync.dma_start(out=wq0, in_=Wqr[:, :, 0:256])

                    xT_sb = xT_pool.tile([P, NDC, T], BF16)    # 16 KB/part
                    xTre = xT.rearrange("(c p) t -> p c t", p=P)
                    for dch in range(4):  # chunked so compute starts early
                        nc.sync.dma_start(
                            out=xT_sb[:, 2 * dch : 2 * dch + 2, :],
                            in_=xTre[:, 2 * dch : 2 * dch + 2, :],
                        )

                    for name, W in (("q", Wq), ("k", Wk), ("v", Wv)):
                        Wr = W.rearrange("(c p) o -> p c o", p=P)
                        for oc in range(4):  # 256-wide output chunks
                            if name == "q" and oc == 0:
                                w_t = wq0
                            else:
                                w_t = wload.tile(
                                    [P, NDC, 256], BF16, tag="wchunk"
                                )
                                nc.sync.dma_start(
                                    out=w_t, in_=Wr[:, :, ts(oc, 256)]
                                )
                            if name == "v":
                                for tk in range(NTK):
                                    ps = qkv_ps.tile([P, 256], F32, tag="v")
                                    for dc in range(NDC):
                                        nc.tensor.matmul(
                                            ps,
                                            lhsT=xT_sb[:, dc, ts(tk, P)],
                                            rhs=w_t[:, dc, :],
                                            start=(dc == 0),
                                            stop=(dc == NDC - 1),
                                        )
                                    nc.vector.tensor_copy(
                                        out=v_sb[:, tk, oc * 4 : oc * 4 + 4, 0:DK],
                                        in_=ps.rearrange("p (h d) -> p h d", d=DK),
                                    )
                            elif name == "q":
                                for ot in range(2):  # 128-wide tiles in chunk
                                    o_tile = oc * 2 + ot
                                    ps = qkv_ps.tile([P, TQ], F32, tag="qk")
                                    for dc in range(NDC):
                                        nc.tensor.matmul(
                                            ps,
                                            lhsT=w_t[:, dc, ts(ot, P)],
                                            rhs=xT_sb[:, dc, 0:TQ],
                                            start=(dc == 0),
                                            stop=(dc == NDC - 1),
                                        )
                                    for i in range(2):  # head halves
                                        h = 2 * o_tile + i
                                        hp = i * DK
                                        nc.vector.tensor_copy(
                                            out=qT_pad[hp : hp + DK, h, :],
                                            in_=ps[hp : hp + DK, :],
                                        )
                            else:
                                for ot in range(2):
                                    o_tile = oc * 2 + ot
                                    for tci in range(2):
                                        ps = qkv_ps.tile([P, TQ], F32, tag="qk")
                                        for dc in range(NDC):
                                            nc.tensor.matmul(
                                                ps,
                                                lhsT=w_t[:, dc, ts(ot, P)],
                                                rhs=xT_sb[:, dc, ts(tci, 512)],
                                                start=(dc == 0),
                                                stop=(dc == NDC - 1),
                                            )
                                        nc.vector.tensor_copy(
                                            out=kT_sb[:, o_tile, ts(tci, 512)],
                                            in_=ps,
                                        )

                # ---- FFN weight streams: queue-mode rings mean these pools
                # get fresh addresses; the GpSimd-issued DMAs below run
                # during attention / FFN-h with no false dependencies.
                with (
                    tc.tile_pool(name="c2_sb", bufs=1) as c2_sb,
                    tc.tile_pool(name="w1load", bufs=3) as w1load,
                    tc.tile_pool(name="w2load", bufs=6) as w2load,
                ):
                    b1_sb = c2_sb.tile([P, NDF], F32)
                    nc.sync.dma_start(
                        out=b1_sb, in_=b1p.rearrange("(c p) -> p c", p=P)
                    )
                    g2_bc = c2_sb.tile([P, D], F32)
                    nc.sync.dma_start(out=g2_bc, in_=bcast(g2, D))
                    be2_bc = c2_sb.tile([P, D], F32)
                    nc.sync.dma_start(out=be2_bc, in_=bcast(be2, D))

                    W1r = W1.rearrange("(c p) f -> p c f", p=P)
                    w1_tiles = []
                    for fc in range(8):  # 512-wide df chunks
                        w_t = w1load.tile(
                            [P, NDC, 512], BF16, tag="w1chunk", name=f"w1_{fc}"
                        )
                        nc.gpsimd.dma_start(out=w_t, in_=W1r[:, :, ts(fc, 512)])
                        w1_tiles.append(w_t)
                    W2r = W2.rearrange("(c p) o -> p c o", p=P)
                    w2_tiles = []
                    for blk in range(16):  # 2 df-tiles per W2 block
                        w_t = w2load.tile(
                            [P, 2, D], BF16, tag="w2chunk", name=f"w2_{blk}"
                        )
                        nc.scalar.dma_start(
                            out=w_t, in_=W2r[:, blk * 2 : blk * 2 + 2, :]
                        )
                        w2_tiles.append(w_t)

                    # ---- Phase B: attention ----
                    with (
                        tc.tile_pool(name="pt_pool", bufs=2) as pt_pool,
                        tc.tile_pool(name="attT_pool", bufs=2) as attT_pool,
                        tc.tile_pool(name="st_ps", bufs=4, space="PSUM") as st_ps,
                        tc.tile_pool(name="att_ps", bufs=2, space="PSUM") as att_ps,
                        tc.tile_pool(name="tr_ps", bufs=2, space="PSUM") as tr_ps,
                    ):
                        for h in range(H):
                            hc = h // 2  # feature chunk of kT
                            pt = pt_pool.tile([P, NTK, TQ], BF16, tag="pt")
                            for tk in range(NTK):
                                st = st_ps.tile([P, TQ], F32, tag="st")
                                # kT holds the head pair; qT_pad is zero on
                                # the other head's partitions -> K=128
                                nc.tensor.matmul(
                                    st,
                                    lhsT=kT_sb[:, hc, ts(tk, P)],
                                    rhs=qT_pad[:, h, :],
                                    start=True,
                                    stop=True,
                                )
                                nc.scalar.activation(
                                    out=pt[:, tk, :], in_=st,
                                    func=AF.Exp, scale=0.125,
                                )
                            aps = att_ps.tile([DK + 1, TQ], F32, tag="attT")
                            for tk in range(NTK):
                                nc.tensor.matmul(
                                    aps,
                                    lhsT=v_sb[:, tk, h, :],
                                    rhs=pt[:, tk, :],
                                    start=(tk == 0),
                                    stop=(tk == NTK - 1),
                                )
                            attT = attT_pool.tile(
                                [DK + 1, TQ], F32, tag="attT_sb"
                            )
                            nc.vector.tensor_copy(out=attT, in_=aps)
                            for q in range(NTQ):
                                trp = tr_ps.tile([P, DK + 1], F32, tag="tr")
                                nc.tensor.transpose(
                                    trp,
                                    attT[:, ts(q, P)],
                                    ident[: DK + 1, : DK + 1],
                                )
                                rec = small.tile([P, 1], F32, tag="rec")
                                nc.vector.reciprocal(
                                    out=rec, in_=trp[:, DK : DK + 1]
                                )
                                nc.vector.tensor_scalar_mul(
                                    out=att_sb[:, q, h, :],
                                    in0=trp[:, 0:DK],
                                    scalar1=rec,
                                )

                    # ============ Phase C1: residual + LN1 + x1T ============
                    with (
                        tc.tile_pool(name="c1_sb", bufs=1) as c1_sb,
                        tc.tile_pool(name="c1_ps", bufs=2, space="PSUM") as c1_ps,
                    ):
                        g1_bc = c1_sb.tile([P, D], F32)
                        nc.sync.dma_start(out=g1_bc, in_=bcast(g1, D))
                        beb_bc = c1_sb.tile([P, D], F32)
                        nc.sync.dma_start(out=beb_bc, in_=bcast(beb, D))

                        xan_sb = c1_sb.tile([P, NTQ, D], F32)
                        nc.sync.dma_start(
                            out=xan_sb,
                            in_=xh.rearrange("(q p) d -> p q d", p=P),
                        )
                        att_flat = att_sb.rearrange("p q h d -> p q (h d)")
                        for q in range(NTQ):
                            nc.vector.tensor_add(
                                out=xan_sb[:, q, :],
                                in0=xan_sb[:, q, :],
                                in1=att_flat[:, q, :],
                            )
                            _ln_normalize_tile(
                                nc, stats, small, xan_sb[:, q, :], eps_sb
                            )
                            for dc in range(NDC):
                                trp = c1_ps.tile([P, P], F32, tag="x1t")
                                nc.tensor.transpose(
                                    trp, xan_sb[:, q, ts(dc, P)], ident
                                )
                                nc.vector.tensor_copy(
                                    out=x1T_sb[:, dc, ts(q, P)], in_=trp
                                )

                        # ================= Phase C2: FFN =================
                        hT_sb = c2_sb.tile([P, NDF, TQ], BF16)  # 32 KB/part
                        with tc.tile_pool(
                            name="h_ps", bufs=4, space="PSUM"
                        ) as h_ps:
                            for fc in range(8):
                                w_t = w1_tiles[fc]
                                for ft in range(4):
                                    df = fc * 4 + ft
                                    ps = h_ps.tile([P, TQ], F32, tag="h")
                                    for dc in range(NDC):
                                        nc.tensor.matmul(
                                            ps,
                                            lhsT=w_t[:, dc, ts(ft, P)],
                                            rhs=x1T_sb[:, dc, :],
                                            start=(dc == 0),
                                            stop=(dc == NDC - 1),
                                        )
                                    nc.scalar.activation(
                                        out=hT_sb[:, df, :],
                                        in_=ps,
                                        func=AF.Relu,
                                        bias=b1_sb[:, df : df + 1],
                                    )

                        # residual term x1+b2 = xan*g1 + beb, off the
                        # critical path (DVE runs during FFN-h matmuls)
                        for q in range(NTQ):
                            nc.vector.tensor_mul(
                                out=x1b_sb[:, q, :],
                                in0=xan_sb[:, q, :],
                                in1=g1_bc,
                            )
                            nc.vector.tensor_add(
                                out=x1b_sb[:, q, :],
                                in0=x1b_sb[:, q, :],
                                in1=beb_bc,
                            )

                    outre = out.rearrange("(q p) d -> p q d", p=P)
                    with tc.tile_pool(
                        name="ff_ps", bufs=1, space="PSUM"
                    ) as ff_ps:
                        ff_tiles = [
                            [
                                ff_ps.tile(
                                    [P, 512], F32,
                                    tag=f"ff_{q}_{oc}", name=f"ff_{q}_{oc}",
                                )
                                for oc in range(2)
                            ]
                            for q in range(NTQ)
                        ]
                        # pass A: df 0..15 for all psum tiles
                        for blk in range(8):
                            w_t = w2_tiles[blk]
                            for fr in range(2):
                                df = blk * 2 + fr
                                for q in range(NTQ):
                                    for oc in range(2):
                                        nc.tensor.matmul(
                                            ff_tiles[q][oc],
                                            lhsT=hT_sb[:, df, ts(q, P)],
                                            rhs=w_t[:, fr, ts(oc, 512)],
                                            start=(df == 0),
                                            stop=False,
                                        )
                        # pass B: df 16..31, q-major so each tile finishes
                        # early and flows into residual+LN2+output DMA
                        for q in range(NTQ):
                            for blk in range(8, 16):
                                w_t = w2_tiles[blk]
                                for fr in range(2):
                                    df = blk * 2 + fr
                                    for oc in range(2):
                                        nc.tensor.matmul(
                                            ff_tiles[q][oc],
                                            lhsT=hT_sb[:, df, ts(q, P)],
                                            rhs=w_t[:, fr, ts(oc, 512)],
                                            start=False,
                                            stop=(df == NDF - 1),
                                        )
                            for oc in range(2):
                                nc.vector.tensor_add(
                                    out=x1b_sb[:, q, ts(oc, 512)],
                                    in0=ff_tiles[q][oc],
                                    in1=x1b_sb[:, q, ts(oc, 512)],
                                )
                            xi = x1b_sb[:, q, :]
                            _ln_normalize_tile(nc, stats, small, xi, eps_sb)
                            nc.vector.tensor_mul(out=xi, in0=xi, in1=g2_bc)
                            nc.vector.tensor_add(out=xi, in0=xi, in1=be2_bc)
                            nc.sync.dma_start(out=outre[:, q, :], in_=xi)

    nc.compile()
    return nc


_NC_CACHE = None


def _get_nc():
    global _NC_CACHE
    if _NC_CACHE is None:
        _NC_CACHE = build_nc()
    return _NC_CACHE


def prepare_shared(Wq, Wk, Wv, W1, b1, W2, b2, ln1_g, ln1_b, ln2_g, ln2_b):
    """Host-side input transforms shared by all cores."""
    f = lambda a: np.asarray(a, np.float64)
    W1g = np.asarray(ln1_g, np.float32)[:, None] * np.asarray(W1, np.float32)
    b1p = (f(b1) + f(ln1_b) @ f(W1)).astype(np.float32)
    beb = (f(ln1_b) + f(b2)).astype(np.float32)
    bf = lambda a: np.ascontiguousarray(np.asarray(a).astype(ml_dtypes.bfloat16))
    return {
        "Wq": bf(Wq), "Wk": bf(Wk), "Wv": bf(Wv),
        "W1": bf(W1g), "W2": bf(W2),
        "b1p": np.ascontiguousarray(b1p),
        "beb": np.ascontiguousarray(beb),
        "ln1_g": np.ascontiguousarray(np.asarray(ln1_g, np.float32)),
        "ln2_g": np.ascontiguousarray(np.asarray(ln2_g, np.float32)),
        "ln2_b": np.ascontiguousarray(np.asarray(ln2_b, np.float32)),
    }


def _core_in_map(x, shared, c):
    b, half = divmod(c, 2)
    xb = x[b]
    sl = slice(half * TQ, (half + 1) * TQ)
    other = slice((1 - half) * TQ, (2 - half) * TQ)
    # query-half columns first; key order is softmax-invariant
    xT_perm = np.concatenate([xb[sl].T, xb[other].T], axis=1)
    return dict(
        shared,
        xT=np.ascontiguousarray(xT_perm.astype(ml_dtypes.bfloat16)),
        xh=np.ascontiguousarray(xb[sl]),
    )


def kernel(x, Wq, Wk, Wv, W1, b1, W2, b2, ln1_g, ln1_b, ln2_g, ln2_b):
    x = np.ascontiguousarray(np.asarray(x, dtype=np.float32))
    shared = prepare_shared(
        Wq, Wk, Wv, W1, b1, W2, b2, ln1_g, ln1_b, ln2_g, ln2_b
    )
    in_maps = [_core_in_map(x, shared, c) for c in range(N_CORES)]
    nc = _get_nc()
    res = run_bass_kernel_spmd(nc, in_maps, core_ids=list(range(N_CORES)))
    out = np.empty((B, T, D), np.float32)
    for c in range(N_CORES):
        b, half = divmod(c, 2)
        out[b, half * TQ : (half + 1) * TQ] = res.results[c]["out"]
    return out
